# revision 4
# baseline (speedup 1.0000x reference)
"""Trainium2 Bass kernel for nn_Block_15066745274698 (GQA attention block).

Computation (B=1, T=4096, C=2048, 16 heads x 128, 4 KV groups):
  qkv = x @ W_attn.T ; split q/k/v ; RoPE(q, k) ; causal GQA attention ;
  out = y @ W_proj.T

Sharding: head-parallel over 8 cores, 2 query heads + their KV group per
core. No collectives: each core computes a partial out^T (its 2 heads
pushed through the matching W_proj columns); the host sums the 8 partials.

Device layout (per core) is transpose-oriented so every matmul contracts
over the partition dim with zero on-device transposes of activations:
  qkv^T (f x t) = W_attn_slice^T.T @ x^T      [via lhsT = W_attn^T tiles]
  S^T   (s x t) = K^T.T @ Q^T                 [scores transposed]
  y^T   (d x t) = V.T @ exp(S^T)              [V transposed once on PE]
  out^T (o x t) = W_proj_slice^T.T @ y^T
Softmax: no max-subtraction (scores bounded ~ +-5), exp on ACT with fused
1/sqrt(128) scale, causal handled by block skipping + gpsimd affine_select
on diagonal blocks.

Denominator: P tiles are summed across s-tiles on the (otherwise idle)
GpSimd engine into a bf16 accumulator, then ONE all-ones [128x128] matmul
per (chunk, head) does the 128-partition reduction -- and because every
lhsT column is 1, every psum row IS the denominator, so the broadcast is
free. This replaces the per-s-tile ones-row matmuls of the previous
version, which burned as much PE streaming time as the AV matmuls
themselves (PE cost is per-column, regardless of output rows). The bf16
accumulator rounding (~0.4%/add) averages down ~sqrt(128) in the fp32
partition reduction, leaving den error ~0.1%.
"""
import sys

sys.path.insert(0, "/opt/trn_rl_repo")
import types

import numpy as np
import ml_dtypes

import concourse.bass as bass
import concourse.mybir as mybir
import concourse.tile as tile
from concourse import bacc
from concourse.bass import ts
from concourse.bass_utils import run_bass_kernel_spmd
from concourse.masks import make_identity

T, C = 4096, 2048
HS = 128
TT = 512                 # t-tile (matmul moving free dim)
NT = T // TT             # 8
NCT = C // 128           # 16 c-tiles
F = 512                  # per-core W_attn rows: 2 q heads + k + v
SCALE = 1.0 / float(np.sqrt(np.float32(HS)))

dt = mybir.dt
FP32 = dt.float32
BF16 = dt.bfloat16
AF = mybir.ActivationFunctionType
ALU = mybir.AluOpType

_cache = {}


def install_ntff_hook_shim():
    """antenv.axon_hooks is missing from this image; register the
    ctypes-based NTFF hook ourselves so trace=True works under axon."""
    if "antenv.axon_hooks" in sys.modules:
        return
    import antenv

    mod = types.ModuleType("antenv.axon_hooks")
    mod._hook = None
    mod.set_axon_ntff_profile_hook = lambda h: setattr(mod, "_hook", h)
    mod.get_axon_ntff_profile_hook = lambda: mod._hook
    sys.modules["antenv.axon_hooks"] = mod
    antenv.axon_hooks = mod
    try:
        from trn_agent_boot.trn_boot import _ntff_profile_via_ctypes

        mod.set_axon_ntff_profile_hook(
            _ntff_profile_via_ctypes("/opt/axon/libaxon_pjrt.so")
        )
    except Exception:
        pass


def _rope(nc, rtmp, cos_sl, sin_sl, src_ps, dst):
    """Rotate-half RoPE: src_ps (128d x TT) psum fp32 -> dst (128 x TT) bf16.
    cos_sl/sin_sl are (128 x TT) fp32 with the 64 rotary rows duplicated.
    Two-input DVE ops need equal base partitions only when BOTH inputs are
    SBUF; src_ps is PSUM, so the rotate-half partition shift is applied on
    the PSUM operand and all SBUF+SBUF pairs stay base-aligned."""
    tcos = rtmp.tile([128, TT], FP32, tag="tcos")
    tsin = rtmp.tile([128, TT], FP32, tag="tsin")
    nc.vector.tensor_mul(tcos, src_ps, cos_sl)
    nc.vector.tensor_mul(tsin[0:64, :], src_ps[64:128, :], sin_sl[0:64, :])
    nc.vector.tensor_mul(tsin[64:128, :], src_ps[0:64, :], sin_sl[64:128, :])
    nc.vector.tensor_sub(dst[0:64, :], tcos[0:64, :], tsin[0:64, :])
    nc.vector.tensor_add(dst[64:128, :], tcos[64:128, :], tsin[64:128, :])


def build(taps=False):
    nc = bacc.Bacc(
        "TRN2", target_bir_lowering=False, debug=False, enable_asserts=False
    )
    xT = nc.dram_tensor("xT", [C, T], BF16, kind="ExternalInput").ap()
    waT = nc.dram_tensor("waT", [C, F], BF16, kind="ExternalInput").ap()
    wpT = nc.dram_tensor("wpT", [2 * HS, C], BF16, kind="ExternalInput").ap()
    cos2 = nc.dram_tensor("cos2", [128, T], FP32, kind="ExternalInput").ap()
    sin2 = nc.dram_tensor("sin2", [128, T], FP32, kind="ExternalInput").ap()
    outT = nc.dram_tensor("outT", [C, T], FP32, kind="ExternalOutput").ap()
    if taps:
        d_qkvT = nc.dram_tensor("d_qkvT", [F, T], FP32, kind="ExternalOutput").ap()
        d_QT = nc.dram_tensor("d_QT", [256, T], BF16, kind="ExternalOutput").ap()
        d_KT = nc.dram_tensor("d_KT", [128, T], BF16, kind="ExternalOutput").ap()
        d_y = nc.dram_tensor("d_y", [256, T], BF16, kind="ExternalOutput").ap()

    xT_r = xT.rearrange("(a p) t -> p a t", p=128)     # [128, 16, 4096]
    waT_r = waT.rearrange("(a p) f -> p a f", p=128)   # [128, 16, 512]
    wpT_r = wpT.rearrange("(a p) o -> p a o", p=128)   # [128, 2, 2048]

    with tile.TileContext(nc) as tc:
        with (
            tc.tile_pool(name="singles", bufs=1) as singles,
            tc.tile_pool(name="xp", bufs=3) as xp,
            tc.tile_pool(name="qp", bufs=2 * NT) as qp,
            tc.tile_pool(name="kp", bufs=NT) as kp,
            tc.tile_pool(name="vp", bufs=4 * NT) as vp,
            tc.tile_pool(name="vstage", bufs=2) as vstage,
            tc.tile_pool(name="pp", bufs=14) as pp,
            tc.tile_pool(name="accp", bufs=2) as accp,
            tc.tile_pool(name="rtmp", bufs=4) as rtmp,
            tc.tile_pool(name="ysb", bufs=5) as ysb,
            tc.tile_pool(name="rbp", bufs=2) as rbp,
            tc.tile_pool(name="osb", bufs=6) as osb,
            tc.tile_pool(name="mm_ps", bufs=3, space="PSUM") as mm_ps,
            tc.tile_pool(name="s_ps", bufs=2, space="PSUM") as s_ps,
            tc.tile_pool(name="y_ps", bufs=2, space="PSUM") as y_ps,
            tc.tile_pool(name="aux_ps", bufs=1, space="PSUM") as aux_ps,
        ):
            # ---- persistent tiles (DMA order matters: the very first qkv
            # matmuls need wa chunk 0 + x chunk 0; cos/sin follow for RoPE;
            # wp is not needed until the first out-projection) ----
            wa_sb = singles.tile([128, NCT, F], BF16)
            xt0 = xp.tile([128, NCT, TT], BF16, tag="xt")
            # first c-chunks as small separate transfers on two queues so the
            # first qkv matmul can start within a couple of microseconds
            nc.sync.dma_start(wa_sb[:, 0:1, :], waT_r[:, 0:1, :])
            nc.scalar.dma_start(xt0[:, 0:1, :], xT_r[:, 0:1, 0:TT])
            nc.sync.dma_start(wa_sb[:, 1:2, :], waT_r[:, 1:2, :])
            nc.scalar.dma_start(xt0[:, 1:2, :], xT_r[:, 1:2, 0:TT])
            nc.sync.dma_start(wa_sb[:, 2:4, :], waT_r[:, 2:4, :])
            nc.scalar.dma_start(xt0[:, 2:4, :], xT_r[:, 2:4, 0:TT])
            for q in range(1, 4):
                nc.sync.dma_start(
                    wa_sb[:, 4 * q:4 * (q + 1), :], waT_r[:, 4 * q:4 * (q + 1), :]
                )
                nc.scalar.dma_start(
                    xt0[:, 4 * q:4 * (q + 1), :],
                    xT_r[:, 4 * q:4 * (q + 1), 0:TT],
                )
            cos_sb = singles.tile([128, T], FP32)
            nc.scalar.dma_start(cos_sb, cos2)
            sin_sb = singles.tile([128, T], FP32)
            nc.scalar.dma_start(sin_sb, sin2)
            wp_sb = singles.tile([128, 2, C], BF16)
            nc.gpsimd.dma_start(wp_sb, wpT_r)
            ident = singles.tile([128, 128], BF16)
            make_identity(nc, ident)
            ones_b = singles.tile([128, 128], BF16)
            nc.vector.memset(ones_b, 1.0)

            q_tiles = [[None] * NT for _ in range(2)]
            k_tiles = [None] * NT
            v_tiles = [None] * (4 * NT)
            y_chunks = [[] for _ in range(NT)]

            def emit_proj(i):
                # out projection for t-chunk i; emitted one t-tile late so
                # attention matmuls are available to fill PE stalls while the
                # DVE/ACT drain copies pace the psum slot rotation
                for oi in range(NCT):
                    op = mm_ps.tile([128, TT], FP32, tag="mm")
                    for cj in range(2):
                        nc.tensor.matmul(
                            op,
                            wp_sb[:, cj, oi * 128:(oi + 1) * 128],
                            y_chunks[i][cj],
                            start=(cj == 0),
                            stop=(cj == 1),
                        )
                    ot = osb.tile([128, TT], FP32, tag="ot")
                    if oi % 2 == 0:
                        nc.vector.tensor_copy(ot, op)
                    else:
                        nc.scalar.copy(ot, op)
                    nc.sync.dma_start(
                        outT[oi * 128:(oi + 1) * 128, ts(i, TT)], ot
                    )

            for i in range(NT):
                # ---- QKV projection for t-chunk i ----
                if i == 0:
                    xt = xt0
                else:
                    xt = xp.tile([128, NCT, TT], BF16, tag="xt")
                    for q in range(4):
                        nc.sync.dma_start(
                            xt[:, 4 * q:4 * (q + 1), :],
                            xT_r[:, 4 * q:4 * (q + 1), ts(i, TT)],
                        )
                for f in range(4):
                    ps = mm_ps.tile([128, TT], FP32, tag="mm")
                    for ci in range(NCT):
                        nc.tensor.matmul(
                            ps,
                            wa_sb[:, ci, f * 128:(f + 1) * 128],
                            xt[:, ci, :],
                            start=(ci == 0),
                            stop=(ci == NCT - 1),
                        )
                    if taps:
                        dbg = osb.tile([128, TT], FP32, tag="dbg")
                        nc.vector.tensor_copy(dbg, ps)
                        nc.sync.dma_start(
                            d_qkvT[f * 128:(f + 1) * 128, ts(i, TT)], dbg
                        )
                    if f < 2:
                        dst = qp.tile([128, TT], BF16, tag="qt")
                        q_tiles[f][i] = dst
                        _rope(nc, rtmp, cos_sb[:, ts(i, TT)],
                              sin_sb[:, ts(i, TT)], ps, dst)
                        if taps:
                            nc.sync.dma_start(
                                d_QT[f * 128:(f + 1) * 128, ts(i, TT)], dst
                            )
                    elif f == 2:
                        dst = kp.tile([128, TT], BF16, tag="kt")
                        k_tiles[i] = dst
                        _rope(nc, rtmp, cos_sb[:, ts(i, TT)],
                              sin_sb[:, ts(i, TT)], ps, dst)
                        if taps:
                            nc.sync.dma_start(d_KT[:, ts(i, TT)], dst)
                    else:
                        vst = vstage.tile([128, TT], BF16, tag="vst")
                        nc.vector.tensor_copy(vst, ps)
                        for j4 in range(4):
                            tp = mm_ps.tile([128, 128], BF16, tag="mm")
                            nc.tensor.transpose(
                                tp, vst[:, j4 * 128:(j4 + 1) * 128], ident
                            )
                            vt = vp.tile([128, 128], BF16, tag="vt")
                            v_tiles[i * 4 + j4] = vt
                            nc.vector.tensor_copy(vt, tp)

                # ---- attention for t-chunk i, both heads ----
                yts = y_chunks[i]
                ns = 4 * (i + 1)
                for h in range(2):
                    yp = y_ps.tile([128, TT], FP32, tag="y")
                    acc = accp.tile([128, TT], BF16, tag="acc")

                    def emit_av(pj, poff, pp_sb):
                        nc.tensor.matmul(
                            yp[:, poff:], v_tiles[pj], pp_sb[:, poff:],
                            start=(pj == 0), stop=(pj == ns - 1),
                            skip_group_check=True,
                        )

                    pend = None
                    for j in range(ns):
                        # diagonal s-tiles: only the causally-valid column
                        # suffix [off:TT) is computed (off = s0 - t0); the
                        # j == 0 matmul always has off == 0, so every psum
                        # column is initialized by the start=True group head
                        off = (j % 4) * 128 if j >= 4 * i else 0
                        sp = s_ps.tile([128, TT], FP32, tag="s")
                        nc.tensor.matmul(
                            sp[:, off:],
                            k_tiles[j // 4][:, (j % 4) * 128:(j % 4 + 1) * 128],
                            q_tiles[h][i][:, off:],
                            start=True,
                            stop=True,
                        )
                        p_sb = pp.tile([128, TT], BF16, tag="p")
                        nc.scalar.activation(
                            p_sb[:, off:], sp[:, off:], AF.Exp, scale=SCALE
                        )
                        if j >= 4 * i:
                            # zero entries with s > t inside the aligned
                            # 128-wide triangle at the start of the slice:
                            # keep iff y - p >= 0 (base 0 after slicing)
                            nc.gpsimd.affine_select(
                                out=p_sb[:, off:],
                                in_=p_sb[:, off:],
                                compare_op=ALU.is_ge,
                                fill=0.0,
                                base=0,
                                pattern=[[1, TT - off]],
                                channel_multiplier=-1,
                            )
                        # denominator partial sums ride the GpSimd engine;
                        # the 128-partition reduction happens in one
                        # all-ones matmul after the loop
                        if j == 0:
                            nc.gpsimd.tensor_copy(acc, p_sb)
                        else:
                            nc.gpsimd.tensor_add(
                                acc[:, off:], acc[:, off:], p_sb[:, off:]
                            )
                        # software pipeline: AV for the previous s-tile is
                        # emitted AFTER this s-tile's score matmul, so the
                        # PE program order never blocks on exp[j] with the
                        # next independent score matmul behind it
                        if pend is not None:
                            emit_av(*pend)
                        pend = (j, off, p_sb)
                    emit_av(*pend)
                    # one matmul: reduce acc over partitions AND broadcast
                    # (every lhsT column is 1 -> every psum row is den)
                    bp = aux_ps.tile([128, TT], FP32, tag="den")
                    nc.tensor.matmul(bp, ones_b, acc, start=True, stop=True)
                    rb = rbp.tile([128, TT], FP32, tag="rb")
                    nc.vector.reciprocal_approx_fast(out=rb, in_=bp)
                    yt = ysb.tile([128, TT], BF16, tag="yt")
                    nc.vector.tensor_mul(yt, yp, rb)
                    yts.append(yt)
                    if taps:
                        nc.sync.dma_start(
                            d_y[h * 128:(h + 1) * 128, ts(i, TT)], yt
                        )

                if i > 0:
                    emit_proj(i - 1)
            emit_proj(NT - 1)

    nc.compile()
    return nc


def _prep_inputs(x, cos, sin, W_attn, W_proj):
    bf = ml_dtypes.bfloat16
    x = np.asarray(x, dtype=np.float32)
    cos = np.asarray(cos, dtype=np.float32)
    sin = np.asarray(sin, dtype=np.float32)
    W_attn = np.asarray(W_attn, dtype=np.float32)
    W_proj = np.asarray(W_proj, dtype=np.float32)

    xT = np.ascontiguousarray(x.reshape(T, C).T).astype(bf)
    cos2 = np.ascontiguousarray(np.concatenate([cos.T, cos.T], axis=0))
    sin2 = np.ascontiguousarray(np.concatenate([sin.T, sin.T], axis=0))

    in_maps = []
    for core in range(8):
        g = core // 2
        qoff = g * 768 + (core % 2) * 256
        rows = np.concatenate(
            [
                W_attn[qoff:qoff + 256],
                W_attn[g * 768 + 512:g * 768 + 640],
                W_attn[g * 768 + 640:g * 768 + 768],
            ],
            axis=0,
        )
        waT = np.ascontiguousarray(rows.T).astype(bf)
        h0 = g * 4 + (core % 2) * 2
        wpT = np.ascontiguousarray(W_proj[:, h0 * 128:h0 * 128 + 256].T).astype(bf)
        in_maps.append(
            {"xT": xT, "waT": waT, "wpT": wpT, "cos2": cos2, "sin2": sin2}
        )
    return in_maps


def kernel(x, cos, sin, W_attn, W_proj, _trace=False, _trace_cores=None):
    if "nc" not in _cache:
        _cache["nc"] = build()
    nc = _cache["nc"]
    in_maps = _prep_inputs(x, cos, sin, W_attn, W_proj)
    kwargs = {}
    if _trace:
        install_ntff_hook_shim()
        kwargs = dict(trace=True, trace_cores=_trace_cores or [0])
    res = run_bass_kernel_spmd(nc, in_maps, core_ids=list(range(8)), **kwargs)
    acc = np.zeros((C, T), dtype=np.float32)
    for r in res.results:
        acc += r["outT"]
    out = np.ascontiguousarray(acc.T).reshape(1, T, C)
    _cache["last_results"] = res
    return out


# revision 7
# speedup vs baseline: 1.2799x; 1.2799x over previous
"""Trainium2 Bass kernel for nn_Block_15066745274698 (GQA attention block).

Computation (B=1, T=4096, C=2048, 16 heads x 128, 4 KV groups):
  qkv = x @ W_attn.T ; split q/k/v ; RoPE(q, k) ; causal GQA attention ;
  out = y @ W_proj.T

Sharding: head-parallel over 8 cores, 2 query heads + their KV group per
core. No collectives: each core computes a partial out^T (its 2 heads
pushed through the matching W_proj columns); the host sums the 8 partials.

Device layout (per core) is transpose-oriented so every matmul contracts
over the partition dim with zero on-device transposes of activations:
  qkv^T (f x t) = W_attn_slice^T.T @ x^T      [via lhsT = W_attn^T tiles]
  S^T   (s x t) = K^T.T @ Q^T                 [scores transposed]
  y^T   (d x t) = V.T @ exp(S^T)              [V transposed once on PE]
  out^T (o x t) = W_proj_slice^T.T @ y^T
Softmax: no max-subtraction (scores bounded ~ +-5), exp on ACT with fused
1/sqrt(128) scale, causal handled by block skipping + gpsimd affine_select
on diagonal blocks.

Denominator: adjacent P s-tiles are pair-summed on the DVE (bf16, 2x
rate), then one all-ones [128x128] matmul per pair accumulates the
128-partition reduction in psum -- and because every lhsT column is 1,
every psum row IS the denominator, so the broadcast is free. Versus the
per-s-tile ones-row matmuls this halves the denominator's PE streaming
(PE cost is per-column, regardless of output rows) and removes the
separate broadcast matmul. The bf16 pair-sum rounding averages down
~sqrt(128) in the fp32 partition reduction, leaving den error ~0.1%.
"""
import sys

sys.path.insert(0, "/opt/trn_rl_repo")
import types

import numpy as np
import ml_dtypes

import concourse.bass as bass
import concourse.mybir as mybir
import concourse.tile as tile
from concourse import bacc
from concourse.bass import ts
from concourse.bass_utils import run_bass_kernel_spmd
from concourse.masks import make_identity

T, C = 4096, 2048
HS = 128
TT = 512                 # t-tile (matmul moving free dim)
NT = T // TT             # 8
NCT = C // 128           # 16 c-tiles
F = 512                  # per-core W_attn rows: 2 q heads + k + v
SCALE = 1.0 / float(np.sqrt(np.float32(HS)))

dt = mybir.dt
FP32 = dt.float32
BF16 = dt.bfloat16
AF = mybir.ActivationFunctionType
ALU = mybir.AluOpType

_cache = {}


def install_ntff_hook_shim():
    """antenv.axon_hooks is missing from this image; register the
    ctypes-based NTFF hook ourselves so trace=True works under axon."""
    if "antenv.axon_hooks" in sys.modules:
        return
    import antenv

    mod = types.ModuleType("antenv.axon_hooks")
    mod._hook = None
    mod.set_axon_ntff_profile_hook = lambda h: setattr(mod, "_hook", h)
    mod.get_axon_ntff_profile_hook = lambda: mod._hook
    sys.modules["antenv.axon_hooks"] = mod
    antenv.axon_hooks = mod
    try:
        from trn_agent_boot.trn_boot import _ntff_profile_via_ctypes

        mod.set_axon_ntff_profile_hook(
            _ntff_profile_via_ctypes("/opt/axon/libaxon_pjrt.so")
        )
    except Exception:
        pass


def _rope(nc, rtmp, cos_sl, sin_sl, src_ps, dst):
    """Rotate-half RoPE: src_ps (128d x TT) psum fp32 -> dst (128 x TT) bf16.
    cos_sl/sin_sl are (128 x TT) fp32 with the 64 rotary rows duplicated.
    Two-input DVE ops need equal base partitions only when BOTH inputs are
    SBUF; src_ps is PSUM, so the rotate-half partition shift is applied on
    the PSUM operand and all SBUF+SBUF pairs stay base-aligned."""
    tcos = rtmp.tile([128, TT], FP32, tag="tcos")
    tsin = rtmp.tile([128, TT], FP32, tag="tsin")
    nc.vector.tensor_mul(tcos, src_ps, cos_sl)
    nc.vector.tensor_mul(tsin[0:64, :], src_ps[64:128, :], sin_sl[0:64, :])
    nc.vector.tensor_mul(tsin[64:128, :], src_ps[0:64, :], sin_sl[64:128, :])
    nc.vector.tensor_sub(dst[0:64, :], tcos[0:64, :], tsin[0:64, :])
    nc.vector.tensor_add(dst[64:128, :], tcos[64:128, :], tsin[64:128, :])


def build(taps=False):
    nc = bacc.Bacc(
        "TRN2", target_bir_lowering=False, debug=False, enable_asserts=False
    )
    xT = nc.dram_tensor("xT", [C, T], BF16, kind="ExternalInput").ap()
    waT = nc.dram_tensor("waT", [C, F], BF16, kind="ExternalInput").ap()
    wpT = nc.dram_tensor("wpT", [2 * HS, C], BF16, kind="ExternalInput").ap()
    cos2 = nc.dram_tensor("cos2", [128, T], FP32, kind="ExternalInput").ap()
    sin2 = nc.dram_tensor("sin2", [128, T], FP32, kind="ExternalInput").ap()
    outT = nc.dram_tensor("outT", [C, T], FP32, kind="ExternalOutput").ap()
    if taps:
        d_qkvT = nc.dram_tensor("d_qkvT", [F, T], FP32, kind="ExternalOutput").ap()
        d_QT = nc.dram_tensor("d_QT", [256, T], BF16, kind="ExternalOutput").ap()
        d_KT = nc.dram_tensor("d_KT", [128, T], BF16, kind="ExternalOutput").ap()
        d_y = nc.dram_tensor("d_y", [256, T], BF16, kind="ExternalOutput").ap()

    xT_r = xT.rearrange("(a p) t -> p a t", p=128)     # [128, 16, 4096]
    waT_r = waT.rearrange("(a p) f -> p a f", p=128)   # [128, 16, 512]
    wpT_r = wpT.rearrange("(a p) o -> p a o", p=128)   # [128, 2, 2048]

    with tile.TileContext(nc) as tc:
        with (
            tc.tile_pool(name="singles", bufs=1) as singles,
            tc.tile_pool(name="xp", bufs=3) as xp,
            tc.tile_pool(name="qp", bufs=2 * NT) as qp,
            tc.tile_pool(name="kp", bufs=NT) as kp,
            tc.tile_pool(name="vp", bufs=4 * NT) as vp,
            tc.tile_pool(name="vstage", bufs=2) as vstage,
            tc.tile_pool(name="pp", bufs=14) as pp,
            tc.tile_pool(name="pairp", bufs=4) as pairp,
            tc.tile_pool(name="rtmp", bufs=4) as rtmp,
            tc.tile_pool(name="ysb", bufs=5) as ysb,
            tc.tile_pool(name="rbp", bufs=2) as rbp,
            tc.tile_pool(name="osb", bufs=6) as osb,
            tc.tile_pool(name="mm_ps", bufs=3, space="PSUM") as mm_ps,
            tc.tile_pool(name="s_ps", bufs=2, space="PSUM") as s_ps,
            tc.tile_pool(name="y_ps", bufs=2, space="PSUM") as y_ps,
            tc.tile_pool(name="aux_ps", bufs=1, space="PSUM") as aux_ps,
        ):
            # ---- persistent tiles (DMA order matters: the very first qkv
            # matmuls need wa chunk 0 + x chunk 0; cos/sin follow for RoPE;
            # wp is not needed until the first out-projection) ----
            wa_sb = singles.tile([128, NCT, F], BF16)
            xt0 = xp.tile([128, NCT, TT], BF16, tag="xt")
            # first c-chunks as small separate transfers on two queues so the
            # first qkv matmul can start within a couple of microseconds
            nc.sync.dma_start(wa_sb[:, 0:1, :], waT_r[:, 0:1, :])
            nc.scalar.dma_start(xt0[:, 0:1, :], xT_r[:, 0:1, 0:TT])
            nc.sync.dma_start(wa_sb[:, 1:2, :], waT_r[:, 1:2, :])
            nc.scalar.dma_start(xt0[:, 1:2, :], xT_r[:, 1:2, 0:TT])
            nc.sync.dma_start(wa_sb[:, 2:4, :], waT_r[:, 2:4, :])
            nc.scalar.dma_start(xt0[:, 2:4, :], xT_r[:, 2:4, 0:TT])
            for q in range(1, 4):
                nc.sync.dma_start(
                    wa_sb[:, 4 * q:4 * (q + 1), :], waT_r[:, 4 * q:4 * (q + 1), :]
                )
                nc.scalar.dma_start(
                    xt0[:, 4 * q:4 * (q + 1), :],
                    xT_r[:, 4 * q:4 * (q + 1), 0:TT],
                )
            cos_sb = singles.tile([128, T], FP32)
            nc.scalar.dma_start(cos_sb, cos2)
            sin_sb = singles.tile([128, T], FP32)
            nc.scalar.dma_start(sin_sb, sin2)
            wp_sb = singles.tile([128, 2, C], BF16)
            nc.gpsimd.dma_start(wp_sb, wpT_r)
            ident = singles.tile([128, 128], BF16)
            make_identity(nc, ident)
            ones_b = singles.tile([128, 128], BF16)
            nc.vector.memset(ones_b, 1.0)

            q_tiles = [[None] * NT for _ in range(2)]
            k_tiles = [None] * NT
            v_tiles = [None] * (4 * NT)
            y_chunks = [[] for _ in range(NT)]

            def emit_proj(i):
                # out projection for t-chunk i; emitted one t-tile late so
                # attention matmuls are available to fill PE stalls while the
                # DVE/ACT drain copies pace the psum slot rotation
                for oi in range(NCT):
                    op = mm_ps.tile([128, TT], FP32, tag="mm")
                    for cj in range(2):
                        nc.tensor.matmul(
                            op,
                            wp_sb[:, cj, oi * 128:(oi + 1) * 128],
                            y_chunks[i][cj],
                            start=(cj == 0),
                            stop=(cj == 1),
                        )
                    ot = osb.tile([128, TT], FP32, tag="ot")
                    if oi % 2 == 0:
                        nc.vector.tensor_copy(ot, op)
                    else:
                        nc.scalar.copy(ot, op)
                    nc.sync.dma_start(
                        outT[oi * 128:(oi + 1) * 128, ts(i, TT)], ot
                    )

            for i in range(NT):
                # ---- QKV projection for t-chunk i ----
                if i == 0:
                    xt = xt0
                else:
                    xt = xp.tile([128, NCT, TT], BF16, tag="xt")
                    for q in range(4):
                        nc.sync.dma_start(
                            xt[:, 4 * q:4 * (q + 1), :],
                            xT_r[:, 4 * q:4 * (q + 1), ts(i, TT)],
                        )
                for f in range(4):
                    ps = mm_ps.tile([128, TT], FP32, tag="mm")
                    for ci in range(NCT):
                        nc.tensor.matmul(
                            ps,
                            wa_sb[:, ci, f * 128:(f + 1) * 128],
                            xt[:, ci, :],
                            start=(ci == 0),
                            stop=(ci == NCT - 1),
                        )
                    if taps:
                        dbg = osb.tile([128, TT], FP32, tag="dbg")
                        nc.vector.tensor_copy(dbg, ps)
                        nc.sync.dma_start(
                            d_qkvT[f * 128:(f + 1) * 128, ts(i, TT)], dbg
                        )
                    if f < 2:
                        dst = qp.tile([128, TT], BF16, tag="qt")
                        q_tiles[f][i] = dst
                        _rope(nc, rtmp, cos_sb[:, ts(i, TT)],
                              sin_sb[:, ts(i, TT)], ps, dst)
                        if taps:
                            nc.sync.dma_start(
                                d_QT[f * 128:(f + 1) * 128, ts(i, TT)], dst
                            )
                    elif f == 2:
                        dst = kp.tile([128, TT], BF16, tag="kt")
                        k_tiles[i] = dst
                        _rope(nc, rtmp, cos_sb[:, ts(i, TT)],
                              sin_sb[:, ts(i, TT)], ps, dst)
                        if taps:
                            nc.sync.dma_start(d_KT[:, ts(i, TT)], dst)
                    else:
                        vst = vstage.tile([128, TT], BF16, tag="vst")
                        nc.vector.tensor_copy(vst, ps)
                        for j4 in range(4):
                            tp = mm_ps.tile([128, 128], BF16, tag="mm")
                            nc.tensor.transpose(
                                tp, vst[:, j4 * 128:(j4 + 1) * 128], ident
                            )
                            vt = vp.tile([128, 128], BF16, tag="vt")
                            v_tiles[i * 4 + j4] = vt
                            nc.vector.tensor_copy(vt, tp)

                # ---- attention for t-chunk i, both heads ----
                yts = y_chunks[i]
                ns = 4 * (i + 1)
                npair = ns // 2
                for h in range(2):
                    yp = y_ps.tile([128, TT], FP32, tag="y")
                    bp = aux_ps.tile([128, TT], FP32, tag="den")

                    def emit_av(pj, poff, pp_sb):
                        nc.tensor.matmul(
                            yp[:, poff:], v_tiles[pj], pp_sb[:, poff:],
                            start=(pj == 0), stop=(pj == ns - 1),
                            skip_group_check=True,
                        )

                    def emit_den(m, pairt):
                        # denominator: s-tile pairs are pre-summed on the
                        # DVE, so half the matmul streams; the all-ones
                        # lhsT makes every psum row the denominator (the
                        # 128-row broadcast costs nothing -- PE time is
                        # per-column)
                        nc.tensor.matmul(
                            bp, ones_b, pairt,
                            start=(m == 0), stop=(m == npair - 1),
                            skip_group_check=True,
                        )

                    pend = None
                    dq = []
                    p_prev = None
                    for j in range(ns):
                        # diagonal s-tiles: only the causally-valid column
                        # suffix [off:TT) is computed (off = s0 - t0); the
                        # j == 0 matmul always has off == 0, so every psum
                        # column is initialized by the start=True group head
                        off = (j % 4) * 128 if j >= 4 * i else 0
                        sp = s_ps.tile([128, TT], FP32, tag="s")
                        nc.tensor.matmul(
                            sp[:, off:],
                            k_tiles[j // 4][:, (j % 4) * 128:(j % 4 + 1) * 128],
                            q_tiles[h][i][:, off:],
                            start=True,
                            stop=True,
                        )
                        p_sb = pp.tile([128, TT], BF16, tag="p")
                        nc.scalar.activation(
                            p_sb[:, off:], sp[:, off:], AF.Exp, scale=SCALE
                        )
                        if j >= 4 * i:
                            # full-width select: zeroes both the stale
                            # [0:off) prefix (so pair-sums can run full
                            # width) and the in-block triangle:
                            # keep iff y - off - p >= 0
                            nc.gpsimd.affine_select(
                                out=p_sb,
                                in_=p_sb,
                                compare_op=ALU.is_ge,
                                fill=0.0,
                                base=-off,
                                pattern=[[1, TT]],
                                channel_multiplier=-1,
                            )
                        if j % 2 == 1:
                            pairt = pairp.tile([128, TT], BF16, tag="pair")
                            nc.vector.tensor_add(pairt, p_prev, p_sb)
                            dq.append((j // 2, pairt))
                        p_prev = p_sb
                        # software pipeline: AV for the previous s-tile is
                        # emitted AFTER this s-tile's score matmul, so the
                        # PE program order never blocks on exp[j] with the
                        # next independent score matmul behind it
                        if pend is not None:
                            emit_av(*pend)
                            if len(dq) >= 2:
                                emit_den(*dq.pop(0))
                        pend = (j, off, p_sb)
                    emit_av(*pend)
                    for d in dq:
                        emit_den(*d)
                    rb = rbp.tile([128, TT], FP32, tag="rb")
                    nc.vector.reciprocal_approx_fast(out=rb, in_=bp)
                    yt = ysb.tile([128, TT], BF16, tag="yt")
                    nc.vector.tensor_mul(yt, yp, rb)
                    yts.append(yt)
                    if taps:
                        nc.sync.dma_start(
                            d_y[h * 128:(h + 1) * 128, ts(i, TT)], yt
                        )

                if i > 0:
                    emit_proj(i - 1)
            emit_proj(NT - 1)

    nc.compile()
    return nc


def _prep_inputs(x, cos, sin, W_attn, W_proj):
    bf = ml_dtypes.bfloat16
    x = np.asarray(x, dtype=np.float32)
    cos = np.asarray(cos, dtype=np.float32)
    sin = np.asarray(sin, dtype=np.float32)
    W_attn = np.asarray(W_attn, dtype=np.float32)
    W_proj = np.asarray(W_proj, dtype=np.float32)

    xT = np.ascontiguousarray(x.reshape(T, C).T).astype(bf)
    cos2 = np.ascontiguousarray(np.concatenate([cos.T, cos.T], axis=0))
    sin2 = np.ascontiguousarray(np.concatenate([sin.T, sin.T], axis=0))

    in_maps = []
    for core in range(8):
        g = core // 2
        qoff = g * 768 + (core % 2) * 256
        rows = np.concatenate(
            [
                W_attn[qoff:qoff + 256],
                W_attn[g * 768 + 512:g * 768 + 640],
                W_attn[g * 768 + 640:g * 768 + 768],
            ],
            axis=0,
        )
        waT = np.ascontiguousarray(rows.T).astype(bf)
        h0 = g * 4 + (core % 2) * 2
        wpT = np.ascontiguousarray(W_proj[:, h0 * 128:h0 * 128 + 256].T).astype(bf)
        in_maps.append(
            {"xT": xT, "waT": waT, "wpT": wpT, "cos2": cos2, "sin2": sin2}
        )
    return in_maps


def kernel(x, cos, sin, W_attn, W_proj, _trace=False, _trace_cores=None):
    if "nc" not in _cache:
        _cache["nc"] = build()
    nc = _cache["nc"]
    in_maps = _prep_inputs(x, cos, sin, W_attn, W_proj)
    kwargs = {}
    if _trace:
        install_ntff_hook_shim()
        kwargs = dict(trace=True, trace_cores=_trace_cores or [0])
    res = run_bass_kernel_spmd(nc, in_maps, core_ids=list(range(8)), **kwargs)
    acc = np.zeros((C, T), dtype=np.float32)
    for r in res.results:
        acc += r["outT"]
    out = np.ascontiguousarray(acc.T).reshape(1, T, C)
    _cache["last_results"] = res
    return out


# revision 14
# speedup vs baseline: 1.3046x; 1.0193x over previous
"""Trainium2 Bass kernel for nn_Block_15066745274698 (GQA attention block).

Computation (B=1, T=4096, C=2048, 16 heads x 128, 4 KV groups):
  qkv = x @ W_attn.T ; split q/k/v ; RoPE(q, k) ; causal GQA attention ;
  out = y @ W_proj.T

Sharding: head-parallel over 8 cores, 2 query heads + their KV group per
core. No collectives: each core computes a partial out^T (its 2 heads
pushed through the matching W_proj columns); the host sums the 8 partials.

Device layout (per core) is transpose-oriented so every matmul contracts
over the partition dim with zero on-device transposes of activations:
  qkv^T (f x t) = W_attn_slice^T.T @ x^T      [via lhsT = W_attn^T tiles]
  S^T   (s x t) = K^T.T @ Q^T                 [scores transposed]
  y^T   (d x t) = V.T @ exp(S^T)              [V transposed once on PE]
  out^T (o x t) = W_proj_slice^T.T @ y^T
Softmax: no max-subtraction (scores bounded ~ +-5), exp on ACT with fused
1/sqrt(128) scale, causal handled by block skipping + gpsimd affine_select
on diagonal blocks.

Denominator: adjacent P s-tiles are pair-summed on the DVE (bf16, 2x
rate), then one all-ones [128x128] matmul per pair accumulates the
128-partition reduction in psum -- and because every lhsT column is 1,
every psum row IS the denominator, so the broadcast is free. Versus the
per-s-tile ones-row matmuls this halves the denominator's PE streaming
(PE cost is per-column, regardless of output rows) and removes the
separate broadcast matmul. The bf16 pair-sum rounding averages down
~sqrt(128) in the fp32 partition reduction, leaving den error ~0.1%.
"""
import sys

sys.path.insert(0, "/opt/trn_rl_repo")
import types

import numpy as np
import ml_dtypes

import concourse.bass as bass
import concourse.mybir as mybir
import concourse.tile as tile
from concourse import bacc
from concourse.bass import ts
from concourse.bass_utils import run_bass_kernel_spmd
from concourse.masks import make_identity

T, C = 4096, 2048
HS = 128
TT = 512                 # t-tile (matmul moving free dim)
NT = T // TT             # 8
NCT = C // 128           # 16 c-tiles
F = 512                  # per-core W_attn rows: 2 q heads + k + v
SCALE = 1.0 / float(np.sqrt(np.float32(HS)))

dt = mybir.dt
FP32 = dt.float32
BF16 = dt.bfloat16
AF = mybir.ActivationFunctionType
ALU = mybir.AluOpType

_cache = {}


def install_ntff_hook_shim():
    """antenv.axon_hooks is missing from this image; register the
    ctypes-based NTFF hook ourselves so trace=True works under axon."""
    if "antenv.axon_hooks" in sys.modules:
        return
    import antenv

    mod = types.ModuleType("antenv.axon_hooks")
    mod._hook = None
    mod.set_axon_ntff_profile_hook = lambda h: setattr(mod, "_hook", h)
    mod.get_axon_ntff_profile_hook = lambda: mod._hook
    sys.modules["antenv.axon_hooks"] = mod
    antenv.axon_hooks = mod
    try:
        from trn_agent_boot.trn_boot import _ntff_profile_via_ctypes

        mod.set_axon_ntff_profile_hook(
            _ntff_profile_via_ctypes("/opt/axon/libaxon_pjrt.so")
        )
    except Exception:
        pass


def _rope(nc, rtmp, cos_sl, sin_sl, src_ps, dst):
    """Rotate-half RoPE: src_ps (128d x TT) psum fp32 -> dst (128 x TT) bf16.
    cos_sl/sin_sl are (128 x TT) fp32 with the 64 rotary rows duplicated.
    Two-input DVE ops need equal base partitions only when BOTH inputs are
    SBUF; src_ps is PSUM, so the rotate-half partition shift is applied on
    the PSUM operand and all SBUF+SBUF pairs stay base-aligned."""
    tcos = rtmp.tile([128, TT], FP32, tag="tcos")
    tsin = rtmp.tile([128, TT], FP32, tag="tsin")
    nc.vector.tensor_mul(tcos, src_ps, cos_sl)
    nc.vector.tensor_mul(tsin[0:64, :], src_ps[64:128, :], sin_sl[0:64, :])
    nc.vector.tensor_mul(tsin[64:128, :], src_ps[0:64, :], sin_sl[64:128, :])
    nc.vector.tensor_sub(dst[0:64, :], tcos[0:64, :], tsin[0:64, :])
    nc.vector.tensor_add(dst[64:128, :], tcos[64:128, :], tsin[64:128, :])


def build(taps=False):
    nc = bacc.Bacc(
        "TRN2", target_bir_lowering=False, debug=False, enable_asserts=False
    )
    xT = nc.dram_tensor("xT", [C, T], BF16, kind="ExternalInput").ap()
    waT = nc.dram_tensor("waT", [C, F], BF16, kind="ExternalInput").ap()
    wpT = nc.dram_tensor("wpT", [2 * HS, C], BF16, kind="ExternalInput").ap()
    cos2 = nc.dram_tensor("cos2", [128, T], BF16, kind="ExternalInput").ap()
    sin2 = nc.dram_tensor("sin2", [128, T], BF16, kind="ExternalInput").ap()
    outT = nc.dram_tensor("outT", [C, T], FP32, kind="ExternalOutput").ap()
    if taps:
        d_qkvT = nc.dram_tensor("d_qkvT", [F, T], FP32, kind="ExternalOutput").ap()
        d_QT = nc.dram_tensor("d_QT", [256, T], BF16, kind="ExternalOutput").ap()
        d_KT = nc.dram_tensor("d_KT", [128, T], BF16, kind="ExternalOutput").ap()
        d_y = nc.dram_tensor("d_y", [256, T], BF16, kind="ExternalOutput").ap()

    xT_r = xT.rearrange("(a p) t -> p a t", p=128)     # [128, 16, 4096]
    waT_r = waT.rearrange("(a p) f -> p a f", p=128)   # [128, 16, 512]
    wpT_r = wpT.rearrange("(a p) o -> p a o", p=128)   # [128, 2, 2048]

    with tile.TileContext(nc) as tc:
        with (
            tc.tile_pool(name="singles", bufs=1) as singles,
            tc.tile_pool(name="xp", bufs=3) as xp,
            tc.tile_pool(name="qp", bufs=2 * NT) as qp,
            tc.tile_pool(name="kp", bufs=NT) as kp,
            tc.tile_pool(name="vp", bufs=4 * NT) as vp,
            tc.tile_pool(name="vstage", bufs=2) as vstage,
            tc.tile_pool(name="pp", bufs=14) as pp,
            tc.tile_pool(name="pairp", bufs=4) as pairp,
            tc.tile_pool(name="rtmp", bufs=4) as rtmp,
            tc.tile_pool(name="ysb", bufs=5) as ysb,
            tc.tile_pool(name="rbp", bufs=2) as rbp,
            tc.tile_pool(name="osb", bufs=6) as osb,
            tc.tile_pool(name="mm_ps", bufs=3, space="PSUM") as mm_ps,
            tc.tile_pool(name="s_ps", bufs=2, space="PSUM") as s_ps,
            tc.tile_pool(name="y_ps", bufs=2, space="PSUM") as y_ps,
            tc.tile_pool(name="aux_ps", bufs=1, space="PSUM") as aux_ps,
        ):
            # ---- persistent tiles (DMA order matters: the very first qkv
            # matmuls need wa chunk 0 + x chunk 0; cos/sin follow for RoPE;
            # wp is not needed until the first out-projection) ----
            wa_sb = singles.tile([128, NCT, F], BF16)
            xt0 = xp.tile([128, NCT, TT], BF16, tag="xt")
            # first c-chunks as small separate transfers on two queues so the
            # first qkv matmul can start within a couple of microseconds
            nc.sync.dma_start(wa_sb[:, 0:1, :], waT_r[:, 0:1, :])
            nc.scalar.dma_start(xt0[:, 0:1, :], xT_r[:, 0:1, 0:TT])
            nc.sync.dma_start(wa_sb[:, 1:2, :], waT_r[:, 1:2, :])
            nc.scalar.dma_start(xt0[:, 1:2, :], xT_r[:, 1:2, 0:TT])
            nc.sync.dma_start(wa_sb[:, 2:4, :], waT_r[:, 2:4, :])
            nc.scalar.dma_start(xt0[:, 2:4, :], xT_r[:, 2:4, 0:TT])
            for q in range(1, 4):
                nc.sync.dma_start(
                    wa_sb[:, 4 * q:4 * (q + 1), :], waT_r[:, 4 * q:4 * (q + 1), :]
                )
                nc.scalar.dma_start(
                    xt0[:, 4 * q:4 * (q + 1), :],
                    xT_r[:, 4 * q:4 * (q + 1), 0:TT],
                )
            cos_sb = singles.tile([128, T], BF16)
            nc.scalar.dma_start(cos_sb, cos2)
            sin_sb = singles.tile([128, T], BF16)
            nc.scalar.dma_start(sin_sb, sin2)
            wp_sb = singles.tile([128, 2, C], BF16)
            nc.gpsimd.dma_start(wp_sb, wpT_r)
            ident = singles.tile([128, 128], BF16)
            make_identity(nc, ident)
            ones_b = singles.tile([128, 128], BF16)
            nc.vector.memset(ones_b, 1.0)

            q_tiles = [[None] * NT for _ in range(2)]
            k_tiles = [None] * NT
            v_tiles = [None] * (4 * NT)
            y_chunks = [[] for _ in range(NT)]

            def emit_proj_oi(ip, oi):
                # one out-projection column tile for t-chunk ip; these are
                # interleaved into the NEXT chunk's attention loop so the
                # in-order PE always has independent matmul work behind the
                # exp-paced attention tiles
                op = mm_ps.tile([128, TT], FP32, tag="mm")
                for cj in range(2):
                    nc.tensor.matmul(
                        op,
                        wp_sb[:, cj, oi * 128:(oi + 1) * 128],
                        y_chunks[ip][cj],
                        start=(cj == 0),
                        stop=(cj == 1),
                        skip_group_check=True,
                    )
                ot = osb.tile([128, TT], FP32, tag="ot")
                if oi % 2 == 0:
                    nc.vector.tensor_copy(ot, op)
                else:
                    nc.scalar.copy(ot, op)
                nc.sync.dma_start(
                    outT[oi * 128:(oi + 1) * 128, ts(ip, TT)], ot
                )

            xt_next = None
            for i in range(NT):
                # ---- QKV projection for t-chunk i ----
                xt = xt0 if i == 0 else xt_next
                if i + 1 < NT:
                    # prefetch the NEXT chunk's x a full chunk early so
                    # qkv(i+1) never waits on HBM
                    xt_next = xp.tile([128, NCT, TT], BF16, tag="xt")
                    for q in range(4):
                        nc.sync.dma_start(
                            xt_next[:, 4 * q:4 * (q + 1), :],
                            xT_r[:, 4 * q:4 * (q + 1), ts(i + 1, TT)],
                        )
                for f in range(4):
                    ps = mm_ps.tile([128, TT], FP32, tag="mm")
                    for ci in range(NCT):
                        nc.tensor.matmul(
                            ps,
                            wa_sb[:, ci, f * 128:(f + 1) * 128],
                            xt[:, ci, :],
                            start=(ci == 0),
                            stop=(ci == NCT - 1),
                        )
                    if taps:
                        dbg = osb.tile([128, TT], FP32, tag="dbg")
                        nc.vector.tensor_copy(dbg, ps)
                        nc.sync.dma_start(
                            d_qkvT[f * 128:(f + 1) * 128, ts(i, TT)], dbg
                        )
                    if f < 2:
                        dst = qp.tile([128, TT], BF16, tag="qt")
                        q_tiles[f][i] = dst
                        _rope(nc, rtmp, cos_sb[:, ts(i, TT)],
                              sin_sb[:, ts(i, TT)], ps, dst)
                        if taps:
                            nc.sync.dma_start(
                                d_QT[f * 128:(f + 1) * 128, ts(i, TT)], dst
                            )
                    elif f == 2:
                        dst = kp.tile([128, TT], BF16, tag="kt")
                        k_tiles[i] = dst
                        _rope(nc, rtmp, cos_sb[:, ts(i, TT)],
                              sin_sb[:, ts(i, TT)], ps, dst)
                        if taps:
                            nc.sync.dma_start(d_KT[:, ts(i, TT)], dst)
                    else:
                        vst = vstage.tile([128, TT], BF16, tag="vst")
                        nc.vector.tensor_copy(vst, ps)
                        for j4 in range(4):
                            tp = mm_ps.tile([128, 128], BF16, tag="mm")
                            nc.tensor.transpose(
                                tp, vst[:, j4 * 128:(j4 + 1) * 128], ident
                            )
                            vt = vp.tile([128, 128], BF16, tag="vt")
                            v_tiles[i * 4 + j4] = vt
                            nc.vector.tensor_copy(vt, tp)

                # ---- attention for t-chunk i, both heads ----
                yts = y_chunks[i]
                ns = 4 * (i + 1)
                npair = ns // 2
                pq = list(range(NCT)) if i > 0 else []
                stride = max(1, (8 * (i + 1)) // NCT)
                it = 0
                for h in range(2):
                    yp = y_ps.tile([128, TT], FP32, tag="y")
                    bp = aux_ps.tile([128, TT], FP32, tag="den")

                    def emit_av(pj, poff, pp_sb):
                        nc.tensor.matmul(
                            yp[:, poff:], v_tiles[pj], pp_sb[:, poff:],
                            start=(pj == 0), stop=(pj == ns - 1),
                            skip_group_check=True,
                        )

                    def emit_den(m, pairt):
                        # denominator: s-tile pairs are pre-summed on the
                        # DVE, so half the matmul streams; the all-ones
                        # lhsT makes every psum row the denominator (the
                        # 128-row broadcast costs nothing -- PE time is
                        # per-column)
                        nc.tensor.matmul(
                            bp, ones_b, pairt,
                            start=(m == 0), stop=(m == npair - 1),
                            skip_group_check=True,
                        )

                    pend = None
                    dq = []
                    p_prev = None
                    for j in range(ns):
                        # diagonal s-tiles: only the causally-valid column
                        # suffix [off:TT) is computed (off = s0 - t0); the
                        # j == 0 matmul always has off == 0, so every psum
                        # column is initialized by the start=True group head
                        off = (j % 4) * 128 if j >= 4 * i else 0
                        sp = s_ps.tile([128, TT], FP32, tag="s")
                        nc.tensor.matmul(
                            sp[:, off:],
                            k_tiles[j // 4][:, (j % 4) * 128:(j % 4 + 1) * 128],
                            q_tiles[h][i][:, off:],
                            start=True,
                            stop=True,
                        )
                        p_sb = pp.tile([128, TT], BF16, tag="p")
                        nc.scalar.activation(
                            p_sb[:, off:], sp[:, off:], AF.Exp, scale=SCALE
                        )
                        if j >= 4 * i:
                            # full-width select: zeroes both the stale
                            # [0:off) prefix (so pair-sums can run full
                            # width) and the in-block triangle:
                            # keep iff y - off - p >= 0
                            nc.gpsimd.affine_select(
                                out=p_sb,
                                in_=p_sb,
                                compare_op=ALU.is_ge,
                                fill=0.0,
                                base=-off,
                                pattern=[[1, TT]],
                                channel_multiplier=-1,
                            )
                        if j % 2 == 1:
                            pairt = pairp.tile([128, TT], BF16, tag="pair")
                            nc.vector.tensor_add(pairt, p_prev, p_sb)
                            dq.append((j // 2, pairt))
                        p_prev = p_sb
                        # software pipeline: AV for the previous s-tile is
                        # emitted AFTER this s-tile's score matmul, so the
                        # PE program order never blocks on exp[j] with the
                        # next independent score matmul behind it
                        if pend is not None:
                            emit_av(*pend)
                            if len(dq) >= 2:
                                emit_den(*dq.pop(0))
                            it += 1
                            if pq and it % stride == 0:
                                emit_proj_oi(i - 1, pq.pop(0))
                        pend = (j, off, p_sb)
                    emit_av(*pend)
                    for d in dq:
                        emit_den(*d)
                    rb = rbp.tile([128, TT], FP32, tag="rb")
                    nc.vector.reciprocal_approx_fast(out=rb, in_=bp)
                    yt = ysb.tile([128, TT], BF16, tag="yt")
                    nc.vector.tensor_mul(yt, yp, rb)
                    yts.append(yt)
                    if taps:
                        nc.sync.dma_start(
                            d_y[h * 128:(h + 1) * 128, ts(i, TT)], yt
                        )

                for oi in pq:
                    emit_proj_oi(i - 1, oi)
            for oi in range(NCT):
                emit_proj_oi(NT - 1, oi)

    nc.compile()
    return nc


def _prep_inputs(x, cos, sin, W_attn, W_proj):
    bf = ml_dtypes.bfloat16
    x = np.asarray(x, dtype=np.float32)
    cos = np.asarray(cos, dtype=np.float32)
    sin = np.asarray(sin, dtype=np.float32)
    W_attn = np.asarray(W_attn, dtype=np.float32)
    W_proj = np.asarray(W_proj, dtype=np.float32)

    xT = np.ascontiguousarray(x.reshape(T, C).T).astype(bf)
    cos2 = np.ascontiguousarray(np.concatenate([cos.T, cos.T], axis=0)).astype(bf)
    sin2 = np.ascontiguousarray(np.concatenate([sin.T, sin.T], axis=0)).astype(bf)

    in_maps = []
    for core in range(8):
        g = core // 2
        qoff = g * 768 + (core % 2) * 256
        rows = np.concatenate(
            [
                W_attn[qoff:qoff + 256],
                W_attn[g * 768 + 512:g * 768 + 640],
                W_attn[g * 768 + 640:g * 768 + 768],
            ],
            axis=0,
        )
        waT = np.ascontiguousarray(rows.T).astype(bf)
        h0 = g * 4 + (core % 2) * 2
        wpT = np.ascontiguousarray(W_proj[:, h0 * 128:h0 * 128 + 256].T).astype(bf)
        in_maps.append(
            {"xT": xT, "waT": waT, "wpT": wpT, "cos2": cos2, "sin2": sin2}
        )
    return in_maps


def kernel(x, cos, sin, W_attn, W_proj, _trace=False, _trace_cores=None):
    if "nc" not in _cache:
        _cache["nc"] = build()
    nc = _cache["nc"]
    in_maps = _prep_inputs(x, cos, sin, W_attn, W_proj)
    kwargs = {}
    if _trace:
        install_ntff_hook_shim()
        kwargs = dict(trace=True, trace_cores=_trace_cores or [0])
    res = run_bass_kernel_spmd(nc, in_maps, core_ids=list(range(8)), **kwargs)
    acc = np.zeros((C, T), dtype=np.float32)
    for r in res.results:
        acc += r["outT"]
    out = np.ascontiguousarray(acc.T).reshape(1, T, C)
    _cache["last_results"] = res
    return out
